# revision 1
# baseline (speedup 1.0000x reference)
"""Chamfer distance loss on 8 Trainium2 NeuronCores.

Full inputs: points1 [16, 4096, 3], points2 [16, 4096, 3] (fp32).
Output: scalar fp32 loss = (sum(min_m dist) + sum(min_n dist)) / B.

Sharding: data-parallel over batch B=16 -> 2 batches per core on 8 cores.
Each core computes a partial scalar (sum of row-mins + col-mins for its
batches); host sums the 8 partials and divides by B.

Per-batch device algorithm (per core):
  dist[n, m] = |a_n|^2 + |b_m|^2 - 2 a.b  computed as:
    psum = matmul(lhsT=[ax,ay,az,-.5,-.5,-.5], rhs=[bx,by,bz,bx^2,by^2,bz^2])
         = a.b - |b|^2/2                       (K=6, fp32r, N=512 per bank)
    dist = ScalarE Identity((-2)*psum + bias)  bias = |a_n|^2 per partition
  row-min: DVE reduce-min over free dim, col-min: DVE tensor_tensor min
  accumulated across row stripes; final col-min across partitions via
  PE transpose + reduce-min; sums via reduce-add + matmul with ones.
"""

import time

import numpy as np

import concourse.bacc as bacc
import concourse.mybir as mybir
import concourse.tile as tile
from concourse import bass_utils
from concourse.masks import make_identity

N_CORES = 8

f32 = mybir.dt.float32
f32r = mybir.dt.float32r
f16 = mybir.dt.bfloat16
AF = mybir.ActivationFunctionType
ALU = mybir.AluOpType
AX = mybir.AxisListType

_CACHE = {}
last_exec_seconds = None  # wall time of the device dispatch (set per call)

USE_TTR = False  # fused tensor_tensor_reduce row-min (all-zero/err on HW)
USE_T16 = True   # 16-bit PE transposes for the col-min fold
GPS_MOD = 0      # 0 = off (gpsimd TT has no min op — codegen rejects)
ROWMIN_ON = True   # timing attribution knob
COLMIN_ON = True   # timing attribution knob
PSW = 2048       # psum group width (512 | 1024 | 2048)
PS_BUFS = 2      # psum pool bufs (PSW//512 banks each; total <= 8 banks)
W_BF16 = False   # bf16 matmul operands (3e-2 rel err — unusable)
EVAC_ON = True   # timing attribution knob (ScalarE evacuation)
RG = 4           # PE row-groups for concurrent matmuls (1 | 2 | 4)


def _build(bl: int, n: int, m: int, repeat: int = 1):
    """Build the SPMD module for bl batches of [n x 3] vs [m x 3] points.

    repeat > 1 wraps the whole computation in a hardware For_i loop that
    recomputes the same result `repeat` times — used only for timing.
    """
    assert n % 128 == 0 and m % PSW == 0
    n_stripes = n // 128
    n_groups = m // PSW
    wdt = f16 if W_BF16 else f32r

    nc = bacc.Bacc("TRN2", target_bir_lowering=False, debug=False)
    p1 = nc.dram_tensor("p1T", [bl, 3, n], f32, kind="ExternalInput")
    p2 = nc.dram_tensor("p2T", [bl, 3, m], f32, kind="ExternalInput")
    out = nc.dram_tensor("out", [1, bl], f32, kind="ExternalOutput")

    with tile.TileContext(nc) as tc:
        with (
            tc.tile_pool(name="const", bufs=1) as constp,
            tc.tile_pool(name="pts", bufs=2) as ptsp,
            tc.tile_pool(name="sq", bufs=1) as sqp,
            tc.tile_pool(name="stage", bufs=2) as stagep,
            tc.tile_pool(name="acc", bufs=2) as accp,
            tc.tile_pool(name="dist", bufs=4) as distp,
            tc.tile_pool(name="small", bufs=4) as smallp,
            tc.tile_pool(name="psum", bufs=PS_BUFS, space="PSUM") as psump,
        ):
            ident = constp.tile([128, 128], f16 if USE_T16 else f32)
            make_identity(nc, ident[:])
            ones128 = constp.tile([128, 1], f32)
            nc.gpsimd.memset(ones128[:], 1.0)
            ones3 = constp.tile([3, 1], f32)
            nc.gpsimd.memset(ones3[:], 1.0)
            out_sb = constp.tile([1, bl], f32)

            import contextlib
            loop_ctx = (
                tc.For_i(0, repeat, 1) if repeat > 1 else contextlib.nullcontext()
            )
            with loop_ctx:
                for b in range(bl):
                    nrows = 32 * (RG - 1) + 6
                    a6 = ptsp.tile([nrows, n], wdt, tag="a6")
                    b6 = ptsp.tile([nrows, m], wdt, tag="b6")
                    asq = sqp.tile([3, n], f32, tag="asq")
                    bsq = sqp.tile([3, m], f32, tag="bsq")

                    stage_a = stagep.tile([6, n], f32, tag="stage")
                    nc.gpsimd.memset(stage_a[:], -0.5)
                    nc.sync.dma_start(stage_a[0:3, :], p1.ap()[b])
                    nc.gpsimd.tensor_tensor(
                        asq[:], stage_a[0:3, :], stage_a[0:3, :], ALU.mult
                    )
                    nc.vector.tensor_copy(a6[0:6, :], stage_a[:])
                    for rg in range(1, RG):
                        nc.sync.dma_start(a6[32 * rg : 32 * rg + 6, :], a6[0:6, :])

                    stage_b = stagep.tile([6, m], f32, tag="stage")
                    nc.sync.dma_start(stage_b[0:3, :], p2.ap()[b])
                    nc.gpsimd.tensor_tensor(
                        bsq[:], stage_b[0:3, :], stage_b[0:3, :], ALU.mult
                    )
                    nc.sync.dma_start(stage_b[3:6, :], bsq[:])
                    nc.vector.tensor_copy(b6[0:6, :], stage_b[:])
                    for rg in range(1, RG):
                        nc.sync.dma_start(b6[32 * rg : 32 * rg + 6, :], b6[0:6, :])

                    # |a_n|^2 as per-partition bias columns: a2c[p, s] for stripe s
                    a2psum = psump.tile([128, PSW], f32, tag="mm")
                    for t in range(n_stripes):
                        nc.tensor.matmul(
                            a2psum[:, t : t + 1],
                            asq[:, 128 * t : 128 * (t + 1)],
                            ones3[:],
                            start=True,
                            stop=True,
                        )
                    a2c = smallp.tile([128, n_stripes], f32, tag="a2c")
                    nc.vector.tensor_copy(a2c[:], a2psum[:, 0:n_stripes])

                    acc = accp.tile([128, m], f16, tag="acc")
                    use_gps = GPS_MOD > 0 and n_stripes > 2
                    acc2 = None
                    if use_gps:
                        acc2 = accp.tile([128, m], f16, tag="acc2")
                    gps_seen = [False] * n_groups
                    dve_seen = [False] * n_groups
                    rowmin = smallp.tile([128, n_stripes], f16, tag="rowmin")
                    if not ROWMIN_ON:
                        nc.vector.memset(rowmin[:], 0.0)
                    for s in range(n_stripes):
                        on_gps = use_gps and (s % GPS_MOD == GPS_MOD - 1)
                        ssl = slice(128 * s, 128 * (s + 1))
                        dts = []
                        for g in range(n_groups):
                            ps = psump.tile([128, PSW], f32, tag="mm")
                            for j in range(PSW // 512):
                                mo = PSW * g + 512 * j
                                ro = 32 * ((g * (PSW // 512) + j) % RG)
                                nc.tensor.matmul(
                                    ps[:, 512 * j : 512 * (j + 1)],
                                    a6[ro : ro + 6, ssl],
                                    b6[ro : ro + 6, mo : mo + 512],
                                    start=True,
                                    stop=True,
                                    tile_position=(ro, 0),
                                )
                            gsl = slice(PSW * g, PSW * (g + 1))
                            dt_ = distp.tile([128, PSW], f16, tag=f"dist{g}")
                            if EVAC_ON:
                                nc.scalar.activation(
                                    dt_[:], ps[:], AF.Identity,
                                    bias=a2c[:, s : s + 1], scale=-2.0,
                                )
                            elif ROWMIN_ON or COLMIN_ON:
                                nc.vector.memset(dt_[:], 1.0)
                            dts.append(dt_)
                            if COLMIN_ON:
                                if not dve_seen[g]:
                                    nc.vector.tensor_copy(acc[:, gsl], dt_[:])
                                    dve_seen[g] = True
                                else:
                                    nc.vector.tensor_tensor(
                                        acc[:, gsl], acc[:, gsl], dt_[:], ALU.min
                                    )
                        # fused row-min for the stripe: elementwise min of the two
                        # 2048-wide groups + min-reduce, one DVE instruction.
                        if not ROWMIN_ON:
                            pass
                        else:
                            t01 = distp.tile([128, PSW], f16, tag="t01")
                            if n_groups >= 2:
                                nc.vector.tensor_tensor(
                                    t01[:], dts[0][:], dts[1][:], ALU.min
                                )
                            else:
                                nc.vector.tensor_copy(t01[:], dts[0][:])
                            for g in range(2, n_groups):
                                nc.vector.tensor_tensor(
                                    t01[:], t01[:], dts[g][:], ALU.min
                                )
                            w = PSW // 2
                            while w >= 128:
                                nc.vector.tensor_tensor(
                                    t01[:, 0:w], t01[:, 0:w], t01[:, w : 2 * w],
                                    ALU.min,
                                )
                                w //= 2
                            nc.vector.tensor_reduce(
                                rowmin[:, s : s + 1], t01[:, 0:128], axis=AX.X,
                                op=ALU.min,
                            )

                    # col-min across partitions: 16 transposes per psum tile,
                    # then one strided reduce-min per psum tile.
                    n_blocks = m // 128
                    tdt = f16 if USE_T16 else f32
                    cmin = smallp.tile([128, n_blocks], tdt, tag="cmin")
                    if not COLMIN_ON:
                        nc.vector.memset(cmin[:], 0.0)
                    if COLMIN_ON and use_gps and any(gps_seen):
                        for g in range(n_groups):
                            gsl = slice(2048 * g, 2048 * (g + 1))
                            if dve_seen[g]:
                                nc.vector.tensor_tensor(
                                    acc[:, gsl], acc[:, gsl], acc2[:, gsl],
                                    ALU.min,
                                )
                            else:
                                nc.vector.tensor_copy(acc[:, gsl], acc2[:, gsl])
                    if COLMIN_ON:
                        if USE_T16:
                            acc_t = acc
                        else:
                            acc_t = accp.tile([128, m], f32, tag="acc32")
                            nc.vector.tensor_copy(acc_t[:], acc[:])
                        tpb = (PSW // 128) if USE_T16 else (PSW // 256)
                        for k0 in range(0, n_blocks, tpb):
                            pst = psump.tile(
                                [128, PSW if USE_T16 else PSW // 2], tdt, tag="mm"
                            )
                            kk = min(tpb, n_blocks - k0)
                            for k in range(kk):
                                nc.tensor.transpose(
                                    pst[:, 128 * k : 128 * (k + 1)],
                                    acc_t[:, 128 * (k0 + k) : 128 * (k0 + k + 1)],
                                    ident[:],
                                )
                            nc.vector.tensor_reduce(
                                cmin[:, k0 : k0 + kk],
                                pst[:, 0 : 128 * kk].rearrange(
                                    "p (k x) -> p k x", x=128
                                ),
                                axis=AX.X,
                                op=ALU.min,
                            )

                    rs = smallp.tile([128, 1], f32, tag="rs")
                    cs = smallp.tile([128, 1], f32, tag="cs")
                    nc.vector.tensor_reduce(rs[:], rowmin[:], axis=AX.X, op=ALU.add)
                    nc.vector.tensor_reduce(cs[:], cmin[:], axis=AX.X, op=ALU.add)
                    sc = psump.tile([128, PSW], f32, tag="mm")
                    nc.tensor.matmul(sc[0:1, 0:1], rs[:], ones128[:], start=True, stop=False)
                    nc.tensor.matmul(sc[0:1, 0:1], cs[:], ones128[:], start=False, stop=True)
                    nc.vector.tensor_copy(out_sb[0:1, b : b + 1], sc[0:1, 0:1])

                nc.sync.dma_start(out.ap(), out_sb[:])

    nc.finalize()
    return nc


def kernel(points1, points2):
    global last_exec_seconds
    points1 = np.ascontiguousarray(np.asarray(points1), dtype=np.float32)
    points2 = np.ascontiguousarray(np.asarray(points2), dtype=np.float32)
    btot, n, _ = points1.shape
    m = points2.shape[1]
    bl = btot // N_CORES

    key = (bl, n, m)
    if _CACHE.get("key") != key:
        _CACHE["nc"] = _build(bl, n, m)
        _CACHE["key"] = key
    nc = _CACHE["nc"]

    p1t = np.ascontiguousarray(points1.transpose(0, 2, 1))  # [B, 3, n]
    p2t = np.ascontiguousarray(points2.transpose(0, 2, 1))  # [B, 3, m]
    in_maps = [
        {
            "p1T": p1t[c * bl : (c + 1) * bl],
            "p2T": p2t[c * bl : (c + 1) * bl],
        }
        for c in range(N_CORES)
    ]
    t0 = time.time()
    res = bass_utils.run_bass_kernel_spmd(
        nc, in_maps, core_ids=list(range(N_CORES))
    )
    last_exec_seconds = time.time() - t0

    total = np.float64(0.0)
    for r in res.results:
        total += r["out"].astype(np.float64).sum()
    return np.float32(total / btot)



# revision 7
# speedup vs baseline: 1.1779x; 1.1779x over previous
"""Chamfer distance loss on 8 Trainium2 NeuronCores.

Full inputs: points1 [16, 4096, 3], points2 [16, 4096, 3] (fp32).
Output: scalar fp32 loss = (sum(min_m dist) + sum(min_n dist)) / B.

Sharding: data-parallel over batch B=16 -> 2 batches per core on 8 cores.
Each core computes a partial scalar (sum of row-mins + col-mins for its
batches); host sums the 8 partials and divides by B.

Per-batch device algorithm (per core), v2 (quad-batched DVE):
  dist[n, m] = |a_n|^2 + |b_m|^2 - 2 a.b  computed as:
    psum = matmul(lhsT=[ax,ay,az,-.5,-.5,-.5], rhs=[bx,by,bz,bx^2,by^2,bz^2])
         = a.b - |b|^2/2                       (K=6, fp32r, N=512 per bank)
    dist16 = ScalarE Identity((-2)*psum + bias)  bias = |a_n|^2 per partition
  Stripes (128 rows of n) are processed in QUADS of 4; the bf16 dist tiles
  of a quad live in one ring tile [128, 4, 4096] so the row-min fold tree
  runs as ONE DVE op per level over all 4 stripes ([128, 4, w] 3D APs),
  slashing DVE instruction count (the per-op overhead dominated v1).
  col-min: DVE tensor_tensor min into acc per stripe; final col-min across
  partitions via PE transpose + strided reduce-min; sums via reduce-add +
  matmul with ones.
"""

import time

import numpy as np

import concourse.bacc as bacc
import concourse.mybir as mybir
import concourse.tile as tile
from concourse import bass_utils
from concourse.masks import make_identity

N_CORES = 8

f32 = mybir.dt.float32
f32r = mybir.dt.float32r
f16 = mybir.dt.bfloat16
AF = mybir.ActivationFunctionType
ALU = mybir.AluOpType
AX = mybir.AxisListType

_CACHE = {}
last_exec_seconds = None  # wall time of the device dispatch (set per call)

QUAD = 4         # stripes per quad (ring depth)
PSW = 2048       # psum group width (512 | 1024 | 2048 | 4096)
PS_BUFS = 2      # psum pool bufs (PSW//512 banks each; total <= 8 banks)
RG = 4           # PE row-groups for concurrent matmuls (1 | 2 | 4)
SQ_SE = False    # squares on ScalarE instead of gpsimd


def _build(bl: int, n: int, m: int, repeat: int = 1):
    """Build the SPMD module for bl batches of [n x 3] vs [m x 3] points.

    repeat > 1 wraps the whole computation in a hardware For_i loop that
    recomputes the same result `repeat` times — used only for timing.
    """
    assert n % (128 * QUAD) == 0 and m % PSW == 0
    n_stripes = n // 128
    n_quads = n_stripes // QUAD
    n_groups = m // PSW
    wdt = f32r

    nc = bacc.Bacc("TRN2", target_bir_lowering=False, debug=False)
    p1 = nc.dram_tensor("p1T", [bl, 3, n], f32, kind="ExternalInput")
    p2 = nc.dram_tensor("p2T", [bl, 3, m], f32, kind="ExternalInput")
    out = nc.dram_tensor("out", [1, bl], f32, kind="ExternalOutput")

    with tile.TileContext(nc) as tc:
        with (
            tc.tile_pool(name="const", bufs=1) as constp,
            tc.tile_pool(name="pts", bufs=2) as ptsp,
            tc.tile_pool(name="sq", bufs=1) as sqp,
            tc.tile_pool(name="stage", bufs=2) as stagep,
            tc.tile_pool(name="acc", bufs=1) as accp,
            tc.tile_pool(name="ring", bufs=2) as ringp,
            tc.tile_pool(name="t01", bufs=1) as t01p,
            tc.tile_pool(name="small", bufs=4) as smallp,
            tc.tile_pool(name="psum", bufs=PS_BUFS, space="PSUM") as psump,
        ):
            ident = constp.tile([128, 128], f16)
            make_identity(nc, ident[:])
            ones128 = constp.tile([128, 1], f32)
            nc.gpsimd.memset(ones128[:], 1.0)
            ones3 = constp.tile([3, 1], f32)
            nc.gpsimd.memset(ones3[:], 1.0)
            out_sb = constp.tile([1, bl], f32)

            import contextlib
            loop_ctx = (
                tc.For_i(0, repeat, 1) if repeat > 1 else contextlib.nullcontext()
            )
            with loop_ctx:
                for b in range(bl):
                    nrows = 32 * (RG - 1) + 6
                    a6 = ptsp.tile([nrows, n], wdt, tag="a6")
                    b6 = ptsp.tile([nrows, m], wdt, tag="b6")
                    asq = sqp.tile([3, n], f32, tag="asq")

                    stage_a = stagep.tile([6, n], f32, tag="stage")
                    nc.gpsimd.memset(stage_a[:], -0.5)
                    nc.sync.dma_start(stage_a[0:3, :], p1.ap()[b])
                    if SQ_SE:
                        nc.scalar.activation(
                            asq[:], stage_a[0:3, :], AF.Square, bias=0.0, scale=1.0
                        )
                    else:
                        nc.gpsimd.tensor_tensor(
                            asq[:], stage_a[0:3, :], stage_a[0:3, :], ALU.mult
                        )
                    nc.vector.tensor_copy(a6[0:6, :], stage_a[:])
                    for rg in range(1, RG):
                        nc.sync.dma_start(a6[32 * rg : 32 * rg + 6, :], a6[0:6, :])

                    stage_b = stagep.tile([6, m], f32, tag="stage")
                    nc.sync.dma_start(stage_b[0:3, :], p2.ap()[b])
                    # compute-engine outputs must start at partition 0: square
                    # into a small chunk tile, DMA into stage_b rows 3:6
                    bchunk = m // 4
                    for c in range(4):
                        bsq = sqp.tile([3, bchunk], f32, tag="bsq")
                        csl = slice(bchunk * c, bchunk * (c + 1))
                        nc.scalar.activation(
                            bsq[:], stage_b[0:3, csl], AF.Square,
                            bias=0.0, scale=1.0,
                        )
                        nc.sync.dma_start(stage_b[3:6, csl], bsq[:])
                    nc.vector.tensor_copy(b6[0:6, :], stage_b[:])
                    for rg in range(1, RG):
                        nc.sync.dma_start(b6[32 * rg : 32 * rg + 6, :], b6[0:6, :])

                    # |a_n|^2 as per-partition bias columns: a2c[p, s] for stripe s
                    a2psum = psump.tile([128, PSW], f32, tag="mm")
                    for t in range(n_stripes):
                        nc.tensor.matmul(
                            a2psum[:, t : t + 1],
                            asq[:, 128 * t : 128 * (t + 1)],
                            ones3[:],
                            start=True,
                            stop=True,
                        )
                    a2c = smallp.tile([128, n_stripes], f32, tag="a2c")
                    nc.vector.tensor_copy(a2c[:], a2psum[:, 0:n_stripes])

                    acc = accp.tile([128, m], f16, tag="acc")
                    acc_init = [False] * n_groups
                    rowmin = smallp.tile([128, n_stripes], f16, tag="rowmin")

                    for q in range(n_quads):
                        ring = ringp.tile([128, QUAD, m], f16, tag="ring")
                        t01 = t01p.tile([128, QUAD, m // 2], f16, tag="t01")
                        for si in range(QUAD):
                            s = q * QUAD + si
                            ssl = slice(128 * s, 128 * (s + 1))
                            for g in range(n_groups):
                                ps = psump.tile([128, PSW], f32, tag="mm")
                                for j in range(PSW // 512):
                                    mo = PSW * g + 512 * j
                                    ro = 32 * ((g * (PSW // 512) + j) % RG)
                                    nc.tensor.matmul(
                                        ps[:, 512 * j : 512 * (j + 1)],
                                        a6[ro : ro + 6, ssl],
                                        b6[ro : ro + 6, mo : mo + 512],
                                        start=True,
                                        stop=True,
                                        tile_position=(ro, 0),
                                    )
                                gsl = slice(PSW * g, PSW * (g + 1))
                                nc.scalar.activation(
                                    ring[:, si, gsl], ps[:], AF.Identity,
                                    bias=a2c[:, s : s + 1], scale=-2.0,
                                )
                                # col-min accumulate for this stripe/group
                                if not acc_init[g]:
                                    nc.vector.tensor_copy(
                                        acc[:, gsl], ring[:, si, gsl]
                                    )
                                    acc_init[g] = True
                                else:
                                    nc.vector.tensor_tensor(
                                        acc[:, gsl], acc[:, gsl], ring[:, si, gsl],
                                        ALU.min,
                                    )
                            # per-stripe first fold: m -> m/2 (2048-out op)
                            nc.vector.tensor_tensor(
                                t01[:, si, :], ring[:, si, 0 : m // 2],
                                ring[:, si, m // 2 : m], ALU.min,
                            )
                        # quad-batched fold tree: one op per level over 4 stripes
                        w = m // 4
                        while w >= 128:
                            nc.vector.tensor_tensor(
                                t01[:, :, 0:w], t01[:, :, 0:w], t01[:, :, w : 2 * w],
                                ALU.min,
                            )
                            w //= 2
                        nc.vector.tensor_reduce(
                            rowmin[:, q * QUAD : (q + 1) * QUAD],
                            t01[:, :, 0:128],
                            axis=AX.X,
                            op=ALU.min,
                        )

                    # col-min across partitions: 16 transposes per psum tile,
                    # then one strided reduce-min per psum tile.
                    n_blocks = m // 128
                    cmin = smallp.tile([128, n_blocks], f16, tag="cmin")
                    tpb = PSW // 128
                    for k0 in range(0, n_blocks, tpb):
                        pst = psump.tile([128, PSW], f16, tag="mm")
                        kk = min(tpb, n_blocks - k0)
                        for k in range(kk):
                            nc.tensor.transpose(
                                pst[:, 128 * k : 128 * (k + 1)],
                                acc[:, 128 * (k0 + k) : 128 * (k0 + k + 1)],
                                ident[:],
                            )
                        nc.vector.tensor_reduce(
                            cmin[:, k0 : k0 + kk],
                            pst[:, 0 : 128 * kk].rearrange(
                                "p (k x) -> p k x", x=128
                            ),
                            axis=AX.X,
                            op=ALU.min,
                        )

                    rs = smallp.tile([128, 1], f32, tag="rs")
                    cs = smallp.tile([128, 1], f32, tag="cs")
                    nc.vector.tensor_reduce(rs[:], rowmin[:], axis=AX.X, op=ALU.add)
                    nc.vector.tensor_reduce(cs[:], cmin[:], axis=AX.X, op=ALU.add)
                    sc = psump.tile([128, PSW], f32, tag="mm")
                    nc.tensor.matmul(sc[0:1, 0:1], rs[:], ones128[:], start=True, stop=False)
                    nc.tensor.matmul(sc[0:1, 0:1], cs[:], ones128[:], start=False, stop=True)
                    nc.vector.tensor_copy(out_sb[0:1, b : b + 1], sc[0:1, 0:1])

                nc.sync.dma_start(out.ap(), out_sb[:])

    nc.finalize()
    return nc


def kernel(points1, points2):
    global last_exec_seconds
    points1 = np.ascontiguousarray(np.asarray(points1), dtype=np.float32)
    points2 = np.ascontiguousarray(np.asarray(points2), dtype=np.float32)
    btot, n, _ = points1.shape
    m = points2.shape[1]
    bl = btot // N_CORES

    key = (bl, n, m)
    if _CACHE.get("key") != key:
        _CACHE["nc"] = _build(bl, n, m)
        _CACHE["key"] = key
    nc = _CACHE["nc"]

    p1t = np.ascontiguousarray(points1.transpose(0, 2, 1))  # [B, 3, n]
    p2t = np.ascontiguousarray(points2.transpose(0, 2, 1))  # [B, 3, m]
    in_maps = [
        {
            "p1T": p1t[c * bl : (c + 1) * bl],
            "p2T": p2t[c * bl : (c + 1) * bl],
        }
        for c in range(N_CORES)
    ]
    t0 = time.time()
    res = bass_utils.run_bass_kernel_spmd(
        nc, in_maps, core_ids=list(range(N_CORES))
    )
    last_exec_seconds = time.time() - t0

    total = np.float64(0.0)
    for r in res.results:
        total += r["out"].astype(np.float64).sum()
    return np.float32(total / btot)


# revision 15
# speedup vs baseline: 1.3335x; 1.1321x over previous
"""Chamfer distance loss on 8 Trainium2 NeuronCores.

Full inputs: points1 [16, 4096, 3], points2 [16, 4096, 3] (fp32).
Output: scalar fp32 loss = (sum(min_m dist) + sum(min_n dist)) / B.

Sharding: data-parallel over batch B=16 -> 2 batches per core on 8 cores.
Each core computes a partial scalar (sum of row-mins + col-mins for its
batches); host sums the 8 partials and divides by B.

Per-batch device algorithm (per core), v2 (quad-batched DVE):
  dist[n, m] = |a_n|^2 + |b_m|^2 - 2 a.b  computed as:
    psum = matmul(lhsT=[ax,ay,az,-.5,-.5,-.5], rhs=[bx,by,bz,bx^2,by^2,bz^2])
         = a.b - |b|^2/2                       (K=6, fp32r, N=512 per bank)
    dist16 = ScalarE Identity((-2)*psum + bias)  bias = |a_n|^2 per partition
  Stripes (128 rows of n) are processed in QUADS of 4; the bf16 dist tiles
  of a quad live in one ring tile [128, 4, 4096] so the row-min fold tree
  runs as ONE DVE op per level over all 4 stripes ([128, 4, w] 3D APs),
  slashing DVE instruction count (the per-op overhead dominated v1).
  col-min: DVE tensor_tensor min into acc per stripe; final col-min across
  partitions via PE transpose + strided reduce-min; sums via reduce-add +
  matmul with ones.
"""

import time

import numpy as np

import concourse.bacc as bacc
import concourse.mybir as mybir
import concourse.tile as tile
from concourse import bass_utils
from concourse.masks import make_identity

N_CORES = 8

f32 = mybir.dt.float32
f32r = mybir.dt.float32r
f16 = mybir.dt.bfloat16
AF = mybir.ActivationFunctionType
ALU = mybir.AluOpType
AX = mybir.AxisListType

_CACHE = {}
last_exec_seconds = None  # wall time of the device dispatch (set per call)

QUAD = 4         # stripes per quad (ring depth)
PSW = 2048       # psum group width (512 | 1024 | 2048 | 4096)
PS_BUFS = 2      # psum pool bufs (PSW//512 banks each; total <= 8 banks)
RG = 4           # PE row-groups for concurrent matmuls (1 | 2 | 4)
SQ_SE = False    # squares on ScalarE instead of gpsimd
EVAC_ON = True   # timing attribution: ScalarE evacuation
ROWMIN_ON = True  # timing attribution: t01 + quad fold tree
COLMIN_ON = True  # timing attribution: colacc TTs
DVE_EVAC = 0     # groups per stripe evac'd by DVE tensor_copy instead of SE
                 # (0 | 1; 1 alternates which group per stripe)


def _build(bl: int, n: int, m: int, repeat: int = 1):
    """Build the SPMD module for bl batches of [n x 3] vs [m x 3] points.

    repeat > 1 wraps the whole computation in a hardware For_i loop that
    recomputes the same result `repeat` times — used only for timing.
    """
    assert n % (128 * QUAD) == 0 and m % PSW == 0
    n_stripes = n // 128
    n_quads = n_stripes // QUAD
    n_groups = m // PSW
    wdt = f32r

    nc = bacc.Bacc("TRN2", target_bir_lowering=False, debug=False)
    p1 = nc.dram_tensor("p1T", [bl, 3, n], f32, kind="ExternalInput")
    p2 = nc.dram_tensor("p2T", [bl, 3, m], f32, kind="ExternalInput")
    out = nc.dram_tensor("out", [1, bl], f32, kind="ExternalOutput")

    with tile.TileContext(nc) as tc:
        with (
            tc.tile_pool(name="const", bufs=1) as constp,
            tc.tile_pool(name="pts", bufs=2) as ptsp,
            tc.tile_pool(name="sq", bufs=1) as sqp,
            tc.tile_pool(name="stage", bufs=2) as stagep,
            tc.tile_pool(name="acc", bufs=1) as accp,
            tc.tile_pool(name="ring", bufs=2) as ringp,
            tc.tile_pool(name="t01", bufs=1) as t01p,
            tc.tile_pool(name="small", bufs=4) as smallp,
            tc.tile_pool(name="psum", bufs=PS_BUFS, space="PSUM") as psump,
        ):
            ident = constp.tile([128, 128], f16)
            make_identity(nc, ident[:])
            ones128 = constp.tile([128, 1], f32)
            nc.gpsimd.memset(ones128[:], 1.0)
            ones3 = constp.tile([3, 1], f32)
            nc.gpsimd.memset(ones3[:], 1.0)
            out_sb = constp.tile([1, bl], f32)

            import contextlib
            loop_ctx = (
                tc.For_i(0, repeat, 1) if repeat > 1 else contextlib.nullcontext()
            )
            with loop_ctx:
                for b in range(bl):
                    nrows = 32 * (RG - 1) + 6
                    a6 = ptsp.tile([nrows, n], wdt, tag="a6")
                    b6 = ptsp.tile([nrows, m], wdt, tag="b6")
                    asq = sqp.tile([3, n], f32, tag="asq")

                    stage_a = stagep.tile([6, n], f32, tag="stage")
                    nc.gpsimd.memset(stage_a[:], -0.5)
                    nc.sync.dma_start(stage_a[0:3, :], p1.ap()[b])
                    if SQ_SE:
                        nc.scalar.activation(
                            asq[:], stage_a[0:3, :], AF.Square, bias=0.0, scale=1.0
                        )
                    else:
                        nc.gpsimd.tensor_tensor(
                            asq[:], stage_a[0:3, :], stage_a[0:3, :], ALU.mult
                        )
                    nc.sync.dma_start(a6[0:6, :].bitcast(f32), stage_a[:])
                    for rg in range(1, RG):
                        nc.sync.dma_start(a6[32 * rg : 32 * rg + 6, :], a6[0:6, :])

                    stage_b = stagep.tile([6, m], f32, tag="stage")
                    nc.sync.dma_start(stage_b[0:3, :], p2.ap()[b])
                    # compute-engine outputs must start at partition 0: square
                    # into a small chunk tile, DMA into stage_b rows 3:6
                    bchunk = m // 4
                    for c in range(4):
                        bsq = sqp.tile([3, bchunk], f32, tag="bsq")
                        csl = slice(bchunk * c, bchunk * (c + 1))
                        nc.gpsimd.tensor_tensor(
                            bsq[:], stage_b[0:3, csl], stage_b[0:3, csl], ALU.mult
                        )
                        nc.sync.dma_start(stage_b[3:6, csl], bsq[:])
                    nc.sync.dma_start(b6[0:6, :].bitcast(f32), stage_b[:])
                    for rg in range(1, RG):
                        nc.sync.dma_start(b6[32 * rg : 32 * rg + 6, :], b6[0:6, :])

                    # |a_n|^2 as per-partition bias columns: a2c[p, s] for stripe s
                    a2psum = psump.tile([128, PSW], f32, tag="mm")
                    for t in range(n_stripes):
                        nc.tensor.matmul(
                            a2psum[:, t : t + 1],
                            asq[:, 128 * t : 128 * (t + 1)],
                            ones3[:],
                            start=True,
                            stop=True,
                        )
                    a2c = smallp.tile([128, n_stripes], f32, tag="a2c")
                    nc.vector.tensor_copy(a2c[:], a2psum[:, 0:n_stripes])

                    acc = accp.tile([128, m], f16, tag="acc")
                    rowmin = smallp.tile([128, n_stripes], f16, tag="rowmin")

                    for q in range(n_quads):
                        ring = ringp.tile([128, QUAD, m], f16, tag="ring")
                        t01 = t01p.tile([128, QUAD, m // 2], f16, tag="t01")
                        for si in range(QUAD):
                            s = q * QUAD + si
                            ssl = slice(128 * s, 128 * (s + 1))
                            for g in range(n_groups):
                                ps = psump.tile([128, PSW], f32, tag="mm")
                                for j in range(PSW // 512):
                                    mo = PSW * g + 512 * j
                                    ro = 32 * ((g * (PSW // 512) + j) % RG)
                                    nc.tensor.matmul(
                                        ps[:, 512 * j : 512 * (j + 1)],
                                        a6[ro : ro + 6, ssl],
                                        b6[ro : ro + 6, mo : mo + 512],
                                        start=True,
                                        stop=True,
                                        tile_position=(ro, 0),
                                    )
                                # evacuate in 2048-wide chunks, SE act by
                                # default, DVE tensor_scalar when rebalancing
                                for h in range(PSW // 2048):
                                    ci = g * (PSW // 2048) + h
                                    hsl = slice(2048 * h, 2048 * (h + 1))
                                    rsl = slice(
                                        PSW * g + 2048 * h,
                                        PSW * g + 2048 * (h + 1),
                                    )
                                    on_dve = (
                                        DVE_EVAC == 2 and ci == 1
                                    ) or (DVE_EVAC == 1 and ci == 1 and s % 2 == 0)
                                    if not EVAC_ON:
                                        nc.vector.memset(ring[:, si, rsl], 1.0)
                                    elif on_dve:
                                        # DVE evac: dist = (ps * -2) + a2
                                        nc.vector.tensor_scalar(
                                            ring[:, si, rsl], ps[:, hsl],
                                            -2.0, a2c[:, s : s + 1],
                                            op0=ALU.mult, op1=ALU.add,
                                        )
                                    else:
                                        nc.scalar.activation(
                                            ring[:, si, rsl], ps[:, hsl],
                                            AF.Identity,
                                            bias=a2c[:, s : s + 1], scale=-2.0,
                                        )
                            # col-min accumulate, 2048-wide chunks (DVE ops
                            # wider than 2048 hit a slow path)
                            if COLMIN_ON:
                                for c in range(m // 2048):
                                    csl = slice(2048 * c, 2048 * (c + 1))
                                    if s == 0:
                                        nc.vector.tensor_copy(
                                            acc[:, csl], ring[:, si, csl]
                                        )
                                    else:
                                        nc.vector.tensor_tensor(
                                            acc[:, csl], acc[:, csl],
                                            ring[:, si, csl], ALU.min,
                                        )
                            # per-stripe first fold: m -> m/2 (2048-out op)
                            if ROWMIN_ON:
                                nc.vector.tensor_tensor(
                                    t01[:, si, :], ring[:, si, 0 : m // 2],
                                    ring[:, si, m // 2 : m], ALU.min,
                                )
                        # quad-batched fold tree: one op per level over 4 stripes
                        w = m // 4
                        while w >= 128:
                            nc.vector.tensor_tensor(
                                t01[:, :, 0:w], t01[:, :, 0:w], t01[:, :, w : 2 * w],
                                ALU.min,
                            )
                            w //= 2
                        nc.vector.tensor_reduce(
                            rowmin[:, q * QUAD : (q + 1) * QUAD],
                            t01[:, :, 0:128],
                            axis=AX.X,
                            op=ALU.min,
                        )

                    # col-min across partitions: 16 transposes per psum tile,
                    # then one strided reduce-min per psum tile.
                    n_blocks = m // 128
                    cmin = smallp.tile([128, n_blocks], f16, tag="cmin")
                    tpb = PSW // 128
                    for k0 in range(0, n_blocks, tpb):
                        pst = psump.tile([128, PSW], f16, tag="mm")
                        kk = min(tpb, n_blocks - k0)
                        for k in range(kk):
                            nc.tensor.transpose(
                                pst[:, 128 * k : 128 * (k + 1)],
                                acc[:, 128 * (k0 + k) : 128 * (k0 + k + 1)],
                                ident[:],
                            )
                        nc.vector.tensor_reduce(
                            cmin[:, k0 : k0 + kk],
                            pst[:, 0 : 128 * kk].rearrange(
                                "p (k x) -> p k x", x=128
                            ),
                            axis=AX.X,
                            op=ALU.min,
                        )

                    rs = smallp.tile([128, 1], f32, tag="rs")
                    cs = smallp.tile([128, 1], f32, tag="cs")
                    nc.vector.tensor_reduce(rs[:], rowmin[:], axis=AX.X, op=ALU.add)
                    nc.vector.tensor_reduce(cs[:], cmin[:], axis=AX.X, op=ALU.add)
                    sc = psump.tile([128, PSW], f32, tag="mm")
                    nc.tensor.matmul(sc[0:1, 0:1], rs[:], ones128[:], start=True, stop=False)
                    nc.tensor.matmul(sc[0:1, 0:1], cs[:], ones128[:], start=False, stop=True)
                    nc.vector.tensor_copy(out_sb[0:1, b : b + 1], sc[0:1, 0:1])

                nc.sync.dma_start(out.ap(), out_sb[:])

    nc.finalize()
    return nc


def kernel(points1, points2):
    global last_exec_seconds
    points1 = np.ascontiguousarray(np.asarray(points1), dtype=np.float32)
    points2 = np.ascontiguousarray(np.asarray(points2), dtype=np.float32)
    btot, n, _ = points1.shape
    m = points2.shape[1]
    bl = btot // N_CORES

    key = (bl, n, m)
    if _CACHE.get("key") != key:
        _CACHE["nc"] = _build(bl, n, m)
        _CACHE["key"] = key
    nc = _CACHE["nc"]

    p1t = np.ascontiguousarray(points1.transpose(0, 2, 1))  # [B, 3, n]
    p2t = np.ascontiguousarray(points2.transpose(0, 2, 1))  # [B, 3, m]
    in_maps = [
        {
            "p1T": p1t[c * bl : (c + 1) * bl],
            "p2T": p2t[c * bl : (c + 1) * bl],
        }
        for c in range(N_CORES)
    ]
    t0 = time.time()
    res = bass_utils.run_bass_kernel_spmd(
        nc, in_maps, core_ids=list(range(N_CORES))
    )
    last_exec_seconds = time.time() - t0

    total = np.float64(0.0)
    for r in res.results:
        total += r["out"].astype(np.float64).sum()
    return np.float32(total / btot)
